# revision 17
# baseline (speedup 1.0000x reference)
"""Cached multi-head attention, head-sharded (tensor-parallel) over 8 NeuronCores.

Per core: 2 of 16 heads. Q/K/V projections with column-sharded weights,
flash-style attention in S^T layout (keys on partitions), partial Wo
projection with row-sharded Wo; partials are summed on the host.
"""
import sys
import types

sys.path.insert(0, "/opt/trn_rl_repo")

# Provide antenv.axon_hooks (missing in this image) so trace=True works.
try:
    import antenv.axon_hooks  # noqa: F401
except ImportError:
    try:
        import antenv
        from trn_agent_boot.trn_boot import _ntff_profile_via_ctypes

        _mod = types.ModuleType("antenv.axon_hooks")
        _hook = _ntff_profile_via_ctypes("/opt/axon/libaxon_pjrt.so")
        _mod.get_axon_ntff_profile_hook = lambda: _hook
        _mod.set_axon_ntff_profile_hook = lambda h: None
        sys.modules["antenv.axon_hooks"] = _mod
        antenv.axon_hooks = _mod
    except Exception:
        pass

import numpy as np
import concourse.bass as bass  # noqa: F401
import concourse.bass_isa as bass_isa
from concourse import bacc
import concourse.mybir as mybir
import concourse.tile as tile
from concourse.bass_utils import run_bass_kernel_spmd

F32 = mybir.dt.float32
F32R = mybir.dt.float32r
F16 = mybir.dt.float16
EXP = mybir.ActivationFunctionType.Exp
MULT = mybir.AluOpType.mult
ADD = mybir.AluOpType.add
COPY_F = mybir.ActivationFunctionType.Copy

P = 128
B = 2
NCORES = 8
HPC = 2              # heads per core
D = 2048             # d_model
DK = 128             # head dim
EC = HPC * DK        # 256 output dims per core
SN = 2048            # new tokens
SP = 2048            # past tokens
STOT = SN + SP       # 4096 total keys
DT = D // P          # 16 d-tiles
SCH = 512            # phase-1 s-chunk
NSC = SN // SCH      # 8 chunks per batch
QCH = 512            # q chunk
NQC = SN // QCH      # 4
NKT = STOT // P      # 32 k tiles
SCALE = float(1.0 / np.sqrt(DK))

_CACHED_NC = None


def _build():
    nc = bacc.Bacc("TRN2", target_bir_lowering=False, debug=False, num_devices=NCORES)

    xT = nc.dram_tensor("xT", [B, D, SN], F16, kind="ExternalInput")
    wqT = nc.dram_tensor("wqT", [D, EC], F16, kind="ExternalInput")
    wkT = nc.dram_tensor("wkT", [D, EC], F16, kind="ExternalInput")
    wvT = nc.dram_tensor("wvT", [D, EC], F16, kind="ExternalInput")
    woT = nc.dram_tensor("woT", [EC, D], F16, kind="ExternalInput")
    # packed per-k-tile records: [.., kt, 128, 0:128]=K^T tile, [.., 128:256]=V tile
    pkv = nc.dram_tensor("pkv", [B, HPC, SP // P, P, 2 * DK], F16, kind="ExternalInput")
    masks = nc.dram_tensor("masks", [P, 4, QCH], F16, kind="ExternalInput")
    ones_d = nc.dram_tensor("ones", [P, 1], F16, kind="ExternalInput")
    ones32_d = nc.dram_tensor("ones32", [1, 1], F32, kind="ExternalInput")
    out = nc.dram_tensor("out", [B, SN, D], F16, kind="ExternalOutput")


    from contextlib import ExitStack
    with tile.TileContext(nc) as tc, ExitStack() as stack:
        cpool = stack.enter_context(tc.tile_pool(name="const", bufs=1))
        qt_pool = stack.enter_context(tc.tile_pool(name="qt", bufs=2))
        kvsb_pool = stack.enter_context(tc.tile_pool(name="kvsb", bufs=2))
        ot_pool = stack.enter_context(tc.tile_pool(name="ot", bufs=1))
        recip_pool = stack.enter_context(tc.tile_pool(name="recip", bufs=2))

        t_wq = cpool.tile([P, DT, EC], F16, tag="wq")
        t_wk = cpool.tile([P, DT, EC], F16, tag="wk")
        t_wv = cpool.tile([P, DT, EC], F16, tag="wv")

        def load_w(t_w, w_d):
            # split 2MB loads into 4 pieces to spread across DMA queues
            wr = w_d.rearrange("(t p) e -> p t e", p=P)
            for piece in range(4):
                nc.sync.dma_start(
                    t_w[:, piece * 4:(piece + 1) * 4, :],
                    wr[:, piece * 4:(piece + 1) * 4, :])

        load_w(t_wq, wqT)
        # created now, DMA'd later (off the startup critical path)
        t_woT = cpool.tile([P, HPC, D], F16, tag="woT")
        t_masks = cpool.tile([P, 4, QCH], F16, tag="masks")
        t_ones = cpool.tile([P, 1], F16, tag="ones")
        t_ones32 = cpool.tile([1, 1], F32, tag="ones32")

        qt = {}
        kv_sb = {}
        ot = {}
        rcp = {}

        def emit_ph1_chunk(b, sc, xpool, pp1, ppv, drip=None):
            """Q/K/V projections for one 512-token chunk of batch b.

            New-token K^T/V tiles are written directly into the SBUF KV
            buffer (no DRAM roundtrip)."""
            s0 = sc * SCH
            kt0 = SP // P + sc * (SCH // P)
            xt = xpool.tile([P, DT, SCH], F16, tag="xt")
            xr = xT[b].rearrange("(t p) s -> p t s", p=P)
            for piece in range(8):
                nc.sync.dma_start(
                    xt[:, piece * 2:(piece + 1) * 2, :],
                    xr[:, piece * 2:(piece + 1) * 2, s0:s0 + SCH])
            for h in range(HPC):
                e0 = h * DK
                psq = pp1.tile([P, SCH], F32, tag="psq")
                for dt in range(DT):
                    nc.tensor.matmul(
                        psq, t_wq[:, dt, e0:e0 + DK], xt[:, dt, :],
                        start=(dt == 0), stop=(dt == DT - 1))
                nc.scalar.copy(qt[b][:, h, s0:s0 + SCH], psq)
                if b == 0 and sc == 0 and h == 0:
                    load_w(t_wk, wkT)
                    load_w(t_wv, wvT)
                psk = pp1.tile([P, SCH // P, P], F32, tag="psq")
                for dt in range(DT):
                    nc.tensor.matmul(
                        psk[:, :, :], t_wk[:, dt, e0:e0 + DK], xt[:, dt, :],
                        start=(dt == 0), stop=(dt == DT - 1))
                nc.scalar.copy(kv_sb[b][:, h, kt0:kt0 + SCH // P, 0:DK], psk)
                if drip:
                    drip()
                    drip()
            for sub in range(SCH // P):
                psv = ppv.tile([P, HPC, DK], F32, tag="psv")
                for dt in range(DT):
                    nc.tensor.matmul(
                        psv[:, :, :], xt[:, dt, sub * P:(sub + 1) * P],
                        t_wv[:, dt, :],
                        start=(dt == 0), stop=(dt == DT - 1))
                nc.scalar.copy(
                    kv_sb[b][:, 0:HPC, kt0 + sub, DK:2 * DK], psv)
                if drip:
                    drip()

        def emit_attn(b):
            """Attention for both heads of batch b (flash-style, qc pairs)."""
            ot[b] = ot_pool.tile([P, HPC, SN], F16, tag="ot", name=f"ot{b}")
            rcp[b] = recip_pool.tile([P, HPC, NQC * NQC], F32, tag="recip", name=f"rcp{b}")
            pending = [None]
            with tc.tile_pool(name=f"pt{b}", bufs=6) as ptpool, \
                 tc.tile_pool(name=f"sm{b}", bufs=6) as smpool, \
                 tc.tile_pool(name=f"ps2_{b}", bufs=2, space="PSUM") as pp2, \
                 tc.tile_pool(name=f"po{b}", bufs=1, space="PSUM") as ppo:
                for h in range(HPC):
                    for pr in range(NQC // 2):
                        qA, qB = 2 * pr, 2 * pr + 1
                        q0 = qA * QCH
                        kts = list(range(SP // P + 4 * qB + 4))
                        po = ppo.tile([P, 2 * QCH], F32, tag="po")
                        acc = smpool.tile([P, 2 * QCH], F16, tag="acc")
                        state = {"fa": True, "fb": True}
                        pend = []

                        def a_valid(kt):
                            return kt < SP // P + 4 * qA + 4

                        def load_ktile(kt):
                            return (kv_sb[b][:, h, kt, 0:P],
                                    kv_sb[b][:, h, kt, DK:2 * DK])

                        def drain(last):
                            kt, ps_s, pt, v_t = pend.pop(0)
                            av = a_valid(kt)
                            ktn_i = kt - SP // P
                            oA, oB = ktn_i - 4 * qA, ktn_i - 4 * qB
                            soA = oA * P if 0 < oA < 4 else 0
                            soB = oB * P if 0 < oB < 4 else 0
                            lo = soA if av else QCH + soB
                            nc.scalar.activation(
                                pt[:, lo:], ps_s[:, lo:], EXP, scale=SCALE)
                            if 0 <= oA < 4:
                                nc.vector.tensor_mul(
                                    pt[:, soA:QCH], pt[:, soA:QCH],
                                    t_masks[:, oA, soA:])
                            if 0 <= oB < 4:
                                nc.vector.tensor_mul(
                                    pt[:, QCH + soB:], pt[:, QCH + soB:],
                                    t_masks[:, oB, soB:])
                            lastA = last or kt == SP // P + 4 * qA + 3
                            # accumulate probs for the softmax denominator on
                            # DVE (frees the tensor engine for scores/PV)
                            if av:
                                if state["fa"]:
                                    nc.vector.tensor_copy(acc, pt)
                                else:
                                    nc.vector.tensor_add(
                                        acc[:, lo:], acc[:, lo:], pt[:, lo:])
                            else:
                                nc.vector.tensor_add(
                                    acc[:, lo:], acc[:, lo:], pt[:, lo:])
                            if av:
                                nc.tensor.matmul(
                                    po[:, soA:QCH], v_t, pt[:, soA:QCH],
                                    start=state["fa"], stop=lastA,
                                    skip_group_check=True)
                                state["fa"] = False
                            nc.tensor.matmul(
                                po[:, QCH + soB:], v_t, pt[:, QCH + soB:],
                                start=state["fb"], stop=last,
                                skip_group_check=True)
                            state["fb"] = False

                        def sfx(kt, qc):
                            o = (kt - SP // P) - 4 * qc
                            return o * P if 0 < o < 4 else 0

                        for kt in kts:
                            if pending[0] is not None:
                                if kt == 3:
                                    pending[0][0]()
                                elif kt == 10:
                                    pending[0][1]()
                                    pending[0] = None
                            kt_t, v_t = load_ktile(kt)
                            ps_s = pp2.tile([P, 2 * QCH], F32, tag="ps_s")
                            pt = ptpool.tile([P, 2 * QCH], F16, tag="pt")
                            soA, soB = sfx(kt, qA), sfx(kt, qB)
                            if a_valid(kt):
                                nc.tensor.matmul(
                                    ps_s[:, soA:QCH], kt_t,
                                    qt[b][:, h, q0 + soA:q0 + QCH],
                                    start=True, stop=True)
                            nc.tensor.matmul(
                                ps_s[:, QCH + soB:], kt_t,
                                qt[b][:, h, q0 + QCH + soB:q0 + 2 * QCH],
                                start=True, stop=True)
                            pend.append((kt, ps_s, pt, v_t))
                            if len(pend) > 1:
                                drain(False)
                        drain(True)

                        nc.scalar.copy(
                            ot[b][:, h, q0:q0 + QCH], po[:, 0:QCH])
                        nc.vector.tensor_copy(
                            ot[b][:, h, q0 + QCH:q0 + 2 * QCH], po[:, QCH:])

                        def make_fin(h=h, pr=pr, acc=acc):
                            st8 = {}

                            def fin_a():
                                ps_sum = ppo.tile([1, 2 * QCH], F32, tag="psum")
                                nc.tensor.matmul(
                                    ps_sum[0:1, 0:QCH], t_ones, acc[:, 0:QCH],
                                    start=True, stop=True)
                                nc.tensor.matmul(
                                    ps_sum[0:1, QCH:], t_ones, acc[:, QCH:],
                                    start=True, stop=True)
                                sb_sum = smpool.tile([1, 2 * QCH], F32, tag="sbsum")
                                nc.vector.tensor_copy(sb_sum, ps_sum)
                                st8["sb"] = sb_sum

                            def fin_b():
                                sb_sum = st8["sb"]
                                pst = ppo.tile([P, 8], F32, tag="psum")
                                for st in range(8):
                                    nc.tensor.matmul(
                                        pst[:, st:st + 1],
                                        sb_sum[0:1, st * P:(st + 1) * P],
                                        t_ones32[0:1, 0:1],
                                        start=True, stop=True)
                                nc.vector.reciprocal(
                                    rcp[b][:, h, pr * 8:(pr + 1) * 8], pst)
                            return fin_a, fin_b

                        pending[0] = make_fin()
                if pending[0] is not None:
                    pending[0][0]()
                    pending[0][1]()
                    pending[0] = None

        def emit_oproj_tile(b, st, ec, opool, ppso):
            """One Wo-projection output tile (with per-head normalisation)."""
            r0 = st * P
            e0 = ec * QCH
            pso0 = ppso.tile([P, QCH], F32, tag="pso")
            nc.tensor.matmul(
                pso0, ot[b][:, 0, r0:r0 + P],
                t_woT[:, 0, e0:e0 + QCH], start=True, stop=True)
            pso1 = ppso.tile([P, QCH], F32, tag="pso")
            nc.tensor.matmul(
                pso1, ot[b][:, 1, r0:r0 + P],
                t_woT[:, 1, e0:e0 + QCH], start=True, stop=True)
            if b == 1 and st >= 14:
                tmpa = opool.tile([P, QCH], F16, tag="tmpa")
                nc.scalar.mul(tmpa, pso0, rcp[b][:, 0, st:st + 1])
                tmpb = opool.tile([P, QCH], F16, tag="tmpb")
                nc.scalar.mul(tmpb, pso1, rcp[b][:, 1, st:st + 1])
                outt = opool.tile([P, QCH], F16, tag="outt")
                nc.gpsimd.tensor_add(outt, tmpa, tmpb)
            else:
                tmp = opool.tile([P, QCH], F32, tag="tmp")
                nc.scalar.mul(tmp, pso0, rcp[b][:, 0, st:st + 1])
                outt = opool.tile([P, QCH], F16, tag="outt")
                nc.vector.scalar_tensor_tensor(
                    outt, pso1, rcp[b][:, 1, st:st + 1], tmp,
                    op0=MULT, op1=ADD)
            nc.sync.dma_start(out[b, r0:r0 + P, e0:e0 + QCH], outt)

        def emit_oproj_group(b, sts, opool, ppso):
            for st in sts:
                for ec in range(D // QCH):
                    emit_oproj_tile(b, st, ec, opool, ppso)

        # ---------------- schedule ----------------
        def emit_ph1(b, xpool, pp1, ppv, drip=None, after_chunk0=None):
            for sc in range(NSC):
                emit_ph1_chunk(b, sc, xpool, pp1, ppv, drip)
                if sc == 0 and after_chunk0 is not None:
                    after_chunk0()
                # prefetch past-KV tiles into SBUF behind this chunk's x loads
                for h in range(HPC):
                    for kt in range(sc * 4, sc * 4 + 4):
                        nc.sync.dma_start(kv_sb[b][:, h, kt, :], pkv[b, h, kt])

        def load_consts():
            nc.sync.dma_start(t_masks, masks[:, :, :])
            nc.sync.dma_start(t_ones, ones_d[:, :])
            nc.sync.dma_start(t_ones32, ones32_d[:, :])

        qt[0] = qt_pool.tile([P, HPC, SN], F16, tag="qt", name="qt0")
        kv_sb[0] = kvsb_pool.tile([P, HPC, NKT, 2 * DK], F16, tag="kvsb", name="kvsb0")
        with tc.tile_pool(name="xt0", bufs=2) as xpool, \
             tc.tile_pool(name="ps1_0", bufs=3, space="PSUM") as pp1, \
             tc.tile_pool(name="pv0", bufs=2, space="PSUM") as ppv:
            emit_ph1(0, xpool, pp1, ppv, after_chunk0=load_consts)
        wor = woT.rearrange("(h p) d -> p h d", p=P)
        for hh in range(HPC):
            for piece in range(2):
                e0p = piece * (D // 2)
                nc.sync.dma_start(
                    t_woT[:, hh, e0p:e0p + D // 2], wor[:, hh, e0p:e0p + D // 2])

        emit_attn(0)

        qt[1] = qt_pool.tile([P, HPC, SN], F16, tag="qt", name="qt1")
        kv_sb[1] = kvsb_pool.tile([P, HPC, NKT, 2 * DK], F16, tag="kvsb", name="kvsb1")
        with tc.tile_pool(name="xt1", bufs=2) as xpool, \
             tc.tile_pool(name="ps1_1", bufs=2, space="PSUM") as pp1, \
             tc.tile_pool(name="pv1", bufs=2, space="PSUM") as ppv, \
             tc.tile_pool(name="os0", bufs=6) as opool, \
             tc.tile_pool(name="pso0", bufs=4, space="PSUM") as ppso:
            tiles0 = iter([(st, ec) for st in range(SN // P)
                           for ec in range(D // QCH)])

            def drip():
                for _ in range(2):
                    t = next(tiles0, None)
                    if t is not None:
                        emit_oproj_tile(0, t[0], t[1], opool, ppso)

            emit_ph1(1, xpool, pp1, ppv, drip)
            for st, ec in tiles0:
                emit_oproj_tile(0, st, ec, opool, ppso)

        emit_attn(1)

        with tc.tile_pool(name="os1", bufs=6) as opool, \
             tc.tile_pool(name="pso1", bufs=8, space="PSUM") as ppso:
            emit_oproj_group(1, range(SN // P), opool, ppso)

    nc.compile()
    return nc


def _get_nc():
    global _CACHED_NC
    if _CACHED_NC is None:
        _CACHED_NC = _build()
    return _CACHED_NC


def pack_kv(pk, pvv):
    # [B, HPC, S, DK] -> [B, HPC, S//P, P, 2*DK]: [.., 0:DK]=K^T tile, [.., DK:]=V tile
    b, hpc, s, dk = pk.shape
    kt = pk.reshape(b, hpc, s // P, P, dk).transpose(0, 1, 2, 4, 3)
    vt = pvv.reshape(b, hpc, s // P, P, dk)
    return np.ascontiguousarray(np.concatenate([kt, vt], axis=4))


def _prep_inputs(x, past_key, past_value, Wq, Wk, Wv, Wo):
    x = np.asarray(x, np.float32)
    past_key = np.asarray(past_key, np.float32)
    past_value = np.asarray(past_value, np.float32)
    Wq = np.asarray(Wq, np.float32)
    Wk = np.asarray(Wk, np.float32)
    Wv = np.asarray(Wv, np.float32)
    Wo = np.asarray(Wo, np.float32)

    xT = np.ascontiguousarray(x.transpose(0, 2, 1)).astype(np.float16)
    i = np.arange(P)[:, None]
    j = np.arange(QCH)[None, :]
    m = np.stack([(j >= i + o * P) for o in range(4)], axis=1).astype(np.float32)
    m = np.ascontiguousarray(m).astype(np.float16)  # [P, 4, QCH]
    ones = np.ones((P, 1), np.float16)
    ones32 = np.ones((1, 1), np.float32)

    in_maps = []
    for c in range(NCORES):
        e0 = c * EC
        hs = slice(c * HPC, (c + 1) * HPC)
        in_maps.append({
            "xT": xT,
            "wqT": np.ascontiguousarray(Wq[e0:e0 + EC, :].T).astype(np.float16),
            "wkT": np.ascontiguousarray(Wk[e0:e0 + EC, :].T).astype(np.float16),
            "wvT": np.ascontiguousarray(Wv[e0:e0 + EC, :].T).astype(np.float16),
            "woT": np.ascontiguousarray(Wo[:, e0:e0 + EC].T).astype(np.float16),
            "pkv": pack_kv(past_key[:, hs], past_value[:, hs]).astype(np.float16),
            "masks": m,
            "ones": ones,
            "ones32": ones32,
        })
    return in_maps


def _run(inputs, trace=False):
    nc = _get_nc()
    in_maps = _prep_inputs(**inputs)
    res = run_bass_kernel_spmd(nc, in_maps, core_ids=list(range(NCORES)), trace=trace)
    total = res.results[0]["out"].astype(np.float32)
    for c in range(1, NCORES):
        total += res.results[c]["out"]
    return total, res


def kernel(x, past_key, past_value, Wq, Wk, Wv, Wo):
    total, _ = _run(dict(x=x, past_key=past_key, past_value=past_value,
                         Wq=Wq, Wk=Wk, Wv=Wv, Wo=Wo))
    return total



# revision 18
# speedup vs baseline: 1.2246x; 1.2246x over previous
"""Cached multi-head attention, head-sharded (tensor-parallel) over 8 NeuronCores.

Per core: 2 of 16 heads. Q/K/V projections with column-sharded weights,
flash-style attention in S^T layout (keys on partitions), partial Wo
projection with row-sharded Wo; partials are summed on the host.
"""
import sys
import types

sys.path.insert(0, "/opt/trn_rl_repo")

# Provide antenv.axon_hooks (missing in this image) so trace=True works.
try:
    import antenv.axon_hooks  # noqa: F401
except ImportError:
    try:
        import antenv
        from trn_agent_boot.trn_boot import _ntff_profile_via_ctypes

        _mod = types.ModuleType("antenv.axon_hooks")
        _hook = _ntff_profile_via_ctypes("/opt/axon/libaxon_pjrt.so")
        _mod.get_axon_ntff_profile_hook = lambda: _hook
        _mod.set_axon_ntff_profile_hook = lambda h: None
        sys.modules["antenv.axon_hooks"] = _mod
        antenv.axon_hooks = _mod
    except Exception:
        pass

import numpy as np
import concourse.bass as bass  # noqa: F401
import concourse.bass_isa as bass_isa
from concourse import bacc
import concourse.mybir as mybir
import concourse.tile as tile
from concourse.bass_utils import run_bass_kernel_spmd

F32 = mybir.dt.float32
F32R = mybir.dt.float32r
F16 = mybir.dt.float16
EXP = mybir.ActivationFunctionType.Exp
MULT = mybir.AluOpType.mult
ADD = mybir.AluOpType.add
COPY_F = mybir.ActivationFunctionType.Copy

P = 128
B = 2
NCORES = 8
HPC = 2              # heads per core
D = 2048             # d_model
DK = 128             # head dim
EC = HPC * DK        # 256 output dims per core
SN = 2048            # new tokens
SP = 2048            # past tokens
STOT = SN + SP       # 4096 total keys
DT = D // P          # 16 d-tiles
SCH = 512            # phase-1 s-chunk
NSC = SN // SCH      # 8 chunks per batch
QCH = 512            # q chunk
NQC = SN // QCH      # 4
NKT = STOT // P      # 32 k tiles
SCALE = float(1.0 / np.sqrt(DK))

_CACHED_NC = None


def _build():
    nc = bacc.Bacc("TRN2", target_bir_lowering=False, debug=False, num_devices=NCORES)

    xT = nc.dram_tensor("xT", [B, D, SN], F16, kind="ExternalInput")
    wqT = nc.dram_tensor("wqT", [D, EC], F16, kind="ExternalInput")
    wkT = nc.dram_tensor("wkT", [D, EC], F16, kind="ExternalInput")
    wvT = nc.dram_tensor("wvT", [D, EC], F16, kind="ExternalInput")
    woT = nc.dram_tensor("woT", [EC, D], F16, kind="ExternalInput")
    # packed per-k-tile records: [.., kt, 128, 0:128]=K^T tile, [.., 128:256]=V tile
    pkv = nc.dram_tensor("pkv", [B, HPC, SP // P, P, 2 * DK], F16, kind="ExternalInput")
    masks = nc.dram_tensor("masks", [P, 4, QCH], F16, kind="ExternalInput")
    ones_d = nc.dram_tensor("ones", [P, 1], F16, kind="ExternalInput")
    ones32_d = nc.dram_tensor("ones32", [1, 1], F32, kind="ExternalInput")
    out = nc.dram_tensor("out", [B, SN, D], F16, kind="ExternalOutput")


    from contextlib import ExitStack
    with tile.TileContext(nc) as tc, ExitStack() as stack:
        cpool = stack.enter_context(tc.tile_pool(name="const", bufs=1))
        qt_pool = stack.enter_context(tc.tile_pool(name="qt", bufs=2))
        kvsb_pool = stack.enter_context(tc.tile_pool(name="kvsb", bufs=2))
        ot_pool = stack.enter_context(tc.tile_pool(name="ot", bufs=1))
        recip_pool = stack.enter_context(tc.tile_pool(name="recip", bufs=2))

        t_wq = cpool.tile([P, DT, EC], F16, tag="wq")
        t_wk = cpool.tile([P, DT, EC], F16, tag="wk")
        t_wv = cpool.tile([P, DT, EC], F16, tag="wv")

        def load_w(t_w, w_d):
            # split 2MB loads into 4 pieces to spread across DMA queues
            wr = w_d.rearrange("(t p) e -> p t e", p=P)
            for piece in range(4):
                nc.sync.dma_start(
                    t_w[:, piece * 4:(piece + 1) * 4, :],
                    wr[:, piece * 4:(piece + 1) * 4, :])

        load_w(t_wq, wqT)
        # created now, DMA'd later (off the startup critical path)
        t_woT = cpool.tile([P, HPC, D], F16, tag="woT")
        t_masks = cpool.tile([P, 4, QCH], F16, tag="masks")
        t_ones = cpool.tile([P, 1], F16, tag="ones")
        t_ones32 = cpool.tile([1, 1], F32, tag="ones32")

        qt = {}
        kv_sb = {}
        ot = {}
        rcp = {}

        def emit_ph1_chunk(b, sc, xpool, pp1, ppv, drip=None):
            """Q/K/V projections for one 512-token chunk of batch b.

            New-token K^T/V tiles are written directly into the SBUF KV
            buffer (no DRAM roundtrip)."""
            s0 = sc * SCH
            kt0 = SP // P + sc * (SCH // P)
            xt = xpool.tile([P, DT, SCH], F16, tag="xt")
            xr = xT[b].rearrange("(t p) s -> p t s", p=P)
            for piece in range(8):
                nc.sync.dma_start(
                    xt[:, piece * 2:(piece + 1) * 2, :],
                    xr[:, piece * 2:(piece + 1) * 2, s0:s0 + SCH])
            for h in range(HPC):
                e0 = h * DK
                psq = pp1.tile([P, SCH], F32, tag="psq")
                for dt in range(DT):
                    nc.tensor.matmul(
                        psq, t_wq[:, dt, e0:e0 + DK], xt[:, dt, :],
                        start=(dt == 0), stop=(dt == DT - 1))
                nc.scalar.copy(qt[b][:, h, s0:s0 + SCH], psq)
                if b == 0 and sc == 0 and h == 0:
                    load_w(t_wk, wkT)
                    load_w(t_wv, wvT)
                psk = pp1.tile([P, SCH // P, P], F32, tag="psq")
                for dt in range(DT):
                    nc.tensor.matmul(
                        psk[:, :, :], t_wk[:, dt, e0:e0 + DK], xt[:, dt, :],
                        start=(dt == 0), stop=(dt == DT - 1))
                nc.scalar.copy(kv_sb[b][:, h, kt0:kt0 + SCH // P, 0:DK], psk)
                if drip:
                    drip()
                    drip()
            for sub in range(SCH // P):
                psv = ppv.tile([P, HPC, DK], F32, tag="psv")
                for dt in range(DT):
                    nc.tensor.matmul(
                        psv[:, :, :], xt[:, dt, sub * P:(sub + 1) * P],
                        t_wv[:, dt, :],
                        start=(dt == 0), stop=(dt == DT - 1))
                nc.scalar.copy(
                    kv_sb[b][:, 0:HPC, kt0 + sub, DK:2 * DK], psv)
                if drip:
                    drip()

        def emit_attn(b):
            """Attention for both heads of batch b (flash-style, qc pairs)."""
            ot[b] = ot_pool.tile([P, HPC, SN], F16, tag="ot", name=f"ot{b}")
            rcp[b] = recip_pool.tile([P, HPC, NQC * NQC], F32, tag="recip", name=f"rcp{b}")
            pending = [None]
            with tc.tile_pool(name=f"pt{b}", bufs=6) as ptpool, \
                 tc.tile_pool(name=f"sm{b}", bufs=6) as smpool, \
                 tc.tile_pool(name=f"ps2_{b}", bufs=2, space="PSUM") as pp2, \
                 tc.tile_pool(name=f"po{b}", bufs=1, space="PSUM") as ppo:
                for h in range(HPC):
                    for pr in range(NQC // 2):
                        qA, qB = 2 * pr, 2 * pr + 1
                        q0 = qA * QCH
                        kts = list(range(SP // P + 4 * qB + 4))
                        po = ppo.tile([P, 2 * QCH], F32, tag="po")
                        acc = smpool.tile([P, 2 * QCH], F16, tag="acc")
                        state = {"fa": True, "fb": True}
                        pend = []

                        def a_valid(kt):
                            return kt < SP // P + 4 * qA + 4

                        def load_ktile(kt):
                            return (kv_sb[b][:, h, kt, 0:P],
                                    kv_sb[b][:, h, kt, DK:2 * DK])

                        def drain(last):
                            kt, ps_s, pt, v_t = pend.pop(0)
                            av = a_valid(kt)
                            ktn_i = kt - SP // P
                            oA, oB = ktn_i - 4 * qA, ktn_i - 4 * qB
                            soA = oA * P if 0 < oA < 4 else 0
                            soB = oB * P if 0 < oB < 4 else 0
                            lo = soA if av else QCH + soB
                            nc.scalar.activation(
                                pt[:, lo:], ps_s[:, lo:], EXP, scale=SCALE)
                            if 0 <= oA < 4:
                                nc.vector.tensor_mul(
                                    pt[:, soA:QCH], pt[:, soA:QCH],
                                    t_masks[:, oA, soA:])
                            if 0 <= oB < 4:
                                nc.vector.tensor_mul(
                                    pt[:, QCH + soB:], pt[:, QCH + soB:],
                                    t_masks[:, oB, soB:])
                            lastA = last or kt == SP // P + 4 * qA + 3
                            # accumulate probs for the softmax denominator on
                            # DVE (frees the tensor engine for scores/PV)
                            if av:
                                if state["fa"]:
                                    nc.vector.tensor_copy(acc, pt)
                                else:
                                    nc.vector.tensor_add(
                                        acc[:, lo:], acc[:, lo:], pt[:, lo:])
                            else:
                                nc.vector.tensor_add(
                                    acc[:, lo:], acc[:, lo:], pt[:, lo:])
                            if av:
                                nc.tensor.matmul(
                                    po[:, soA:QCH], v_t, pt[:, soA:QCH],
                                    start=state["fa"], stop=lastA,
                                    skip_group_check=True)
                                state["fa"] = False
                            nc.tensor.matmul(
                                po[:, QCH + soB:], v_t, pt[:, QCH + soB:],
                                start=state["fb"], stop=last,
                                skip_group_check=True)
                            state["fb"] = False

                        def sfx(kt, qc):
                            o = (kt - SP // P) - 4 * qc
                            return o * P if 0 < o < 4 else 0

                        for kt in kts:
                            if pending[0] is not None:
                                if kt == 3:
                                    pending[0][0]()
                                elif kt == 10:
                                    pending[0][1]()
                                    pending[0] = None
                            kt_t, v_t = load_ktile(kt)
                            ps_s = pp2.tile([P, 2 * QCH], F32, tag="ps_s")
                            pt = ptpool.tile([P, 2 * QCH], F16, tag="pt")
                            soA, soB = sfx(kt, qA), sfx(kt, qB)
                            if a_valid(kt):
                                nc.tensor.matmul(
                                    ps_s[:, soA:QCH], kt_t,
                                    qt[b][:, h, q0 + soA:q0 + QCH],
                                    start=True, stop=True)
                            nc.tensor.matmul(
                                ps_s[:, QCH + soB:], kt_t,
                                qt[b][:, h, q0 + QCH + soB:q0 + 2 * QCH],
                                start=True, stop=True)
                            pend.append((kt, ps_s, pt, v_t))
                            if len(pend) > 1:
                                drain(False)
                        drain(True)

                        nc.vector.tensor_copy(
                            ot[b][:, h, q0:q0 + 2 * QCH], po)

                        def make_fin(h=h, pr=pr, acc=acc):
                            st8 = {}

                            def fin_a():
                                ps_sum = ppo.tile([1, 2 * QCH], F32, tag="psum")
                                nc.tensor.matmul(
                                    ps_sum[0:1, 0:QCH], t_ones, acc[:, 0:QCH],
                                    start=True, stop=True)
                                nc.tensor.matmul(
                                    ps_sum[0:1, QCH:], t_ones, acc[:, QCH:],
                                    start=True, stop=True)
                                sb_sum = smpool.tile([1, 2 * QCH], F32, tag="sbsum")
                                nc.vector.tensor_copy(sb_sum, ps_sum)
                                st8["sb"] = sb_sum

                            def fin_b():
                                sb_sum = st8["sb"]
                                pst = ppo.tile([P, 8], F32, tag="psum")
                                for st in range(8):
                                    nc.tensor.matmul(
                                        pst[:, st:st + 1],
                                        sb_sum[0:1, st * P:(st + 1) * P],
                                        t_ones32[0:1, 0:1],
                                        start=True, stop=True)
                                nc.vector.reciprocal(
                                    rcp[b][:, h, pr * 8:(pr + 1) * 8], pst)
                            return fin_a, fin_b

                        pending[0] = make_fin()
                if pending[0] is not None:
                    pending[0][0]()
                    pending[0][1]()
                    pending[0] = None

        def emit_oproj_tile(b, st, ec, opool, ppso):
            """One Wo-projection output tile (with per-head normalisation)."""
            r0 = st * P
            e0 = ec * QCH
            pso0 = ppso.tile([P, QCH], F32, tag="pso")
            nc.tensor.matmul(
                pso0, ot[b][:, 0, r0:r0 + P],
                t_woT[:, 0, e0:e0 + QCH], start=True, stop=True)
            pso1 = ppso.tile([P, QCH], F32, tag="pso")
            nc.tensor.matmul(
                pso1, ot[b][:, 1, r0:r0 + P],
                t_woT[:, 1, e0:e0 + QCH], start=True, stop=True)
            tmp = opool.tile([P, QCH], F32, tag="tmp")
            nc.scalar.mul(tmp, pso0, rcp[b][:, 0, st:st + 1])
            outt = opool.tile([P, QCH], F16, tag="outt")
            eng = nc.vector
            eng.scalar_tensor_tensor(
                outt, pso1, rcp[b][:, 1, st:st + 1], tmp,
                op0=MULT, op1=ADD)
            nc.sync.dma_start(out[b, r0:r0 + P, e0:e0 + QCH], outt)

        def emit_oproj_group(b, sts, opool, ppso):
            for st in sts:
                for ec in range(D // QCH):
                    emit_oproj_tile(b, st, ec, opool, ppso)

        # ---------------- schedule ----------------
        def emit_ph1(b, xpool, pp1, ppv, drip=None, after_chunk0=None):
            for sc in range(NSC):
                emit_ph1_chunk(b, sc, xpool, pp1, ppv, drip)
                if sc == 0 and after_chunk0 is not None:
                    after_chunk0()
                # prefetch past-KV tiles into SBUF behind this chunk's x loads
                for h in range(HPC):
                    for kt in range(sc * 4, sc * 4 + 4):
                        nc.sync.dma_start(kv_sb[b][:, h, kt, :], pkv[b, h, kt])

        def load_consts():
            nc.sync.dma_start(t_masks, masks[:, :, :])
            nc.sync.dma_start(t_ones, ones_d[:, :])
            nc.sync.dma_start(t_ones32, ones32_d[:, :])

        qt[0] = qt_pool.tile([P, HPC, SN], F16, tag="qt", name="qt0")
        kv_sb[0] = kvsb_pool.tile([P, HPC, NKT, 2 * DK], F16, tag="kvsb", name="kvsb0")
        with tc.tile_pool(name="xt0", bufs=2) as xpool, \
             tc.tile_pool(name="ps1_0", bufs=3, space="PSUM") as pp1, \
             tc.tile_pool(name="pv0", bufs=2, space="PSUM") as ppv:
            emit_ph1(0, xpool, pp1, ppv, after_chunk0=load_consts)
        wor = woT.rearrange("(h p) d -> p h d", p=P)
        for hh in range(HPC):
            for piece in range(2):
                e0p = piece * (D // 2)
                nc.sync.dma_start(
                    t_woT[:, hh, e0p:e0p + D // 2], wor[:, hh, e0p:e0p + D // 2])

        emit_attn(0)

        qt[1] = qt_pool.tile([P, HPC, SN], F16, tag="qt", name="qt1")
        kv_sb[1] = kvsb_pool.tile([P, HPC, NKT, 2 * DK], F16, tag="kvsb", name="kvsb1")
        with tc.tile_pool(name="xt1", bufs=2) as xpool, \
             tc.tile_pool(name="ps1_1", bufs=2, space="PSUM") as pp1, \
             tc.tile_pool(name="pv1", bufs=2, space="PSUM") as ppv, \
             tc.tile_pool(name="os0", bufs=6) as opool, \
             tc.tile_pool(name="pso0", bufs=4, space="PSUM") as ppso:
            tiles0 = iter([(st, ec) for st in range(SN // P)
                           for ec in range(D // QCH)])

            def drip():
                for _ in range(2):
                    t = next(tiles0, None)
                    if t is not None:
                        emit_oproj_tile(0, t[0], t[1], opool, ppso)

            emit_ph1(1, xpool, pp1, ppv, drip)
            for st, ec in tiles0:
                emit_oproj_tile(0, st, ec, opool, ppso)

        emit_attn(1)

        with tc.tile_pool(name="os1", bufs=6) as opool, \
             tc.tile_pool(name="pso1", bufs=8, space="PSUM") as ppso:
            emit_oproj_group(1, range(SN // P), opool, ppso)

    nc.compile()
    return nc


def _get_nc():
    global _CACHED_NC
    if _CACHED_NC is None:
        _CACHED_NC = _build()
    return _CACHED_NC


def pack_kv(pk, pvv):
    # [B, HPC, S, DK] -> [B, HPC, S//P, P, 2*DK]: [.., 0:DK]=K^T tile, [.., DK:]=V tile
    b, hpc, s, dk = pk.shape
    kt = pk.reshape(b, hpc, s // P, P, dk).transpose(0, 1, 2, 4, 3)
    vt = pvv.reshape(b, hpc, s // P, P, dk)
    return np.ascontiguousarray(np.concatenate([kt, vt], axis=4))


def _prep_inputs(x, past_key, past_value, Wq, Wk, Wv, Wo):
    x = np.asarray(x, np.float32)
    past_key = np.asarray(past_key, np.float32)
    past_value = np.asarray(past_value, np.float32)
    Wq = np.asarray(Wq, np.float32)
    Wk = np.asarray(Wk, np.float32)
    Wv = np.asarray(Wv, np.float32)
    Wo = np.asarray(Wo, np.float32)

    xT = np.ascontiguousarray(x.transpose(0, 2, 1)).astype(np.float16)
    i = np.arange(P)[:, None]
    j = np.arange(QCH)[None, :]
    m = np.stack([(j >= i + o * P) for o in range(4)], axis=1).astype(np.float32)
    m = np.ascontiguousarray(m).astype(np.float16)  # [P, 4, QCH]
    ones = np.ones((P, 1), np.float16)
    ones32 = np.ones((1, 1), np.float32)

    in_maps = []
    for c in range(NCORES):
        e0 = c * EC
        hs = slice(c * HPC, (c + 1) * HPC)
        in_maps.append({
            "xT": xT,
            "wqT": np.ascontiguousarray(Wq[e0:e0 + EC, :].T).astype(np.float16),
            "wkT": np.ascontiguousarray(Wk[e0:e0 + EC, :].T).astype(np.float16),
            "wvT": np.ascontiguousarray(Wv[e0:e0 + EC, :].T).astype(np.float16),
            "woT": np.ascontiguousarray(Wo[:, e0:e0 + EC].T).astype(np.float16),
            "pkv": pack_kv(past_key[:, hs], past_value[:, hs]).astype(np.float16),
            "masks": m,
            "ones": ones,
            "ones32": ones32,
        })
    return in_maps


def _run(inputs, trace=False):
    nc = _get_nc()
    in_maps = _prep_inputs(**inputs)
    res = run_bass_kernel_spmd(nc, in_maps, core_ids=list(range(NCORES)), trace=trace)
    total = res.results[0]["out"].astype(np.float32)
    for c in range(1, NCORES):
        total += res.results[c]["out"]
    return total, res


def kernel(x, past_key, past_value, Wq, Wk, Wv, Wo):
    total, _ = _run(dict(x=x, past_key=past_key, past_value=past_value,
                         Wq=Wq, Wk=Wk, Wv=Wv, Wo=Wo))
    return total



# revision 19
# speedup vs baseline: 1.2301x; 1.0045x over previous
"""Cached multi-head attention, head-sharded (tensor-parallel) over 8 NeuronCores.

Per core: 2 of 16 heads. Q/K/V projections with column-sharded weights,
flash-style attention in S^T layout (keys on partitions), partial Wo
projection with row-sharded Wo; partials are summed on the host.
"""
import sys
import types

sys.path.insert(0, "/opt/trn_rl_repo")

# Provide antenv.axon_hooks (missing in this image) so trace=True works.
try:
    import antenv.axon_hooks  # noqa: F401
except ImportError:
    try:
        import antenv
        from trn_agent_boot.trn_boot import _ntff_profile_via_ctypes

        _mod = types.ModuleType("antenv.axon_hooks")
        _hook = _ntff_profile_via_ctypes("/opt/axon/libaxon_pjrt.so")
        _mod.get_axon_ntff_profile_hook = lambda: _hook
        _mod.set_axon_ntff_profile_hook = lambda h: None
        sys.modules["antenv.axon_hooks"] = _mod
        antenv.axon_hooks = _mod
    except Exception:
        pass

import numpy as np
import concourse.bass as bass  # noqa: F401
import concourse.bass_isa as bass_isa
from concourse import bacc
import concourse.mybir as mybir
import concourse.tile as tile
from concourse.bass_utils import run_bass_kernel_spmd

F32 = mybir.dt.float32
F32R = mybir.dt.float32r
F16 = mybir.dt.float16
EXP = mybir.ActivationFunctionType.Exp
MULT = mybir.AluOpType.mult
ADD = mybir.AluOpType.add
COPY_F = mybir.ActivationFunctionType.Copy

P = 128
B = 2
NCORES = 8
HPC = 2              # heads per core
D = 2048             # d_model
DK = 128             # head dim
EC = HPC * DK        # 256 output dims per core
SN = 2048            # new tokens
SP = 2048            # past tokens
STOT = SN + SP       # 4096 total keys
DT = D // P          # 16 d-tiles
SCH = 512            # phase-1 s-chunk
NSC = SN // SCH      # 8 chunks per batch
QCH = 512            # q chunk
NQC = SN // QCH      # 4
NKT = STOT // P      # 32 k tiles
SCALE = float(1.0 / np.sqrt(DK))

_CACHED_NC = None


def _build():
    nc = bacc.Bacc("TRN2", target_bir_lowering=False, debug=False, num_devices=NCORES)

    xT = nc.dram_tensor("xT", [B, D, SN], F16, kind="ExternalInput")
    wqT = nc.dram_tensor("wqT", [D, EC], F16, kind="ExternalInput")
    wkT = nc.dram_tensor("wkT", [D, EC], F16, kind="ExternalInput")
    wvT = nc.dram_tensor("wvT", [D, EC], F16, kind="ExternalInput")
    woT = nc.dram_tensor("woT", [EC, D], F16, kind="ExternalInput")
    # packed per-k-tile records: [.., kt, 128, 0:128]=K^T tile, [.., 128:256]=V tile
    pkv = nc.dram_tensor("pkv", [B, HPC, SP // P, P, 2 * DK], F16, kind="ExternalInput")
    masks = nc.dram_tensor("masks", [P, 4, QCH], F16, kind="ExternalInput")
    ones_d = nc.dram_tensor("ones", [P, 1], F16, kind="ExternalInput")
    ones32_d = nc.dram_tensor("ones32", [1, 1], F32, kind="ExternalInput")
    out = nc.dram_tensor("out", [B, SN, D], F16, kind="ExternalOutput")


    from contextlib import ExitStack
    with tile.TileContext(nc) as tc, ExitStack() as stack:
        cpool = stack.enter_context(tc.tile_pool(name="const", bufs=1))
        qt_pool = stack.enter_context(tc.tile_pool(name="qt", bufs=2))
        kvsb_pool = stack.enter_context(tc.tile_pool(name="kvsb", bufs=2))
        ot_pool = stack.enter_context(tc.tile_pool(name="ot", bufs=1))
        recip_pool = stack.enter_context(tc.tile_pool(name="recip", bufs=2))

        t_wq = cpool.tile([P, DT, EC], F16, tag="wq")
        t_wk = cpool.tile([P, DT, EC], F16, tag="wk")
        t_wv = cpool.tile([P, DT, EC], F16, tag="wv")

        def load_w(t_w, w_d):
            # split 2MB loads into 4 pieces to spread across DMA queues
            wr = w_d.rearrange("(t p) e -> p t e", p=P)
            for piece in range(4):
                nc.sync.dma_start(
                    t_w[:, piece * 4:(piece + 1) * 4, :],
                    wr[:, piece * 4:(piece + 1) * 4, :])

        load_w(t_wq, wqT)
        # created now, DMA'd later (off the startup critical path)
        t_woT = cpool.tile([P, HPC, D], F16, tag="woT")
        t_masks = cpool.tile([P, 4, QCH], F16, tag="masks")
        t_ones = cpool.tile([P, 1], F16, tag="ones")
        t_ones32 = cpool.tile([1, 1], F32, tag="ones32")

        qt = {}
        kv_sb = {}
        ot = {}
        rcp = {}

        def emit_ph1_chunk(b, sc, xpool, pp1, ppv, drip=None):
            """Q/K/V projections for one 512-token chunk of batch b.

            New-token K^T/V tiles are written directly into the SBUF KV
            buffer (no DRAM roundtrip)."""
            s0 = sc * SCH
            kt0 = SP // P + sc * (SCH // P)
            xt = xpool.tile([P, DT, SCH], F16, tag="xt")
            xr = xT[b].rearrange("(t p) s -> p t s", p=P)
            for piece in range(8):
                nc.sync.dma_start(
                    xt[:, piece * 2:(piece + 1) * 2, :],
                    xr[:, piece * 2:(piece + 1) * 2, s0:s0 + SCH])
            for h in range(HPC):
                e0 = h * DK
                psq = pp1.tile([P, SCH], F32, tag="psq")
                for dt in range(DT):
                    nc.tensor.matmul(
                        psq, t_wq[:, dt, e0:e0 + DK], xt[:, dt, :],
                        start=(dt == 0), stop=(dt == DT - 1))
                nc.scalar.copy(qt[b][:, h, s0:s0 + SCH], psq)
                if b == 0 and sc == 0 and h == 0:
                    load_w(t_wk, wkT)
                    load_w(t_wv, wvT)
                psk = pp1.tile([P, SCH // P, P], F32, tag="psq")
                for dt in range(DT):
                    nc.tensor.matmul(
                        psk[:, :, :], t_wk[:, dt, e0:e0 + DK], xt[:, dt, :],
                        start=(dt == 0), stop=(dt == DT - 1))
                nc.scalar.copy(kv_sb[b][:, h, kt0:kt0 + SCH // P, 0:DK], psk)
                if drip:
                    drip()
                    drip()
            for sub in range(SCH // P):
                psv = ppv.tile([P, HPC, DK], F32, tag="psv")
                for dt in range(DT):
                    nc.tensor.matmul(
                        psv[:, :, :], xt[:, dt, sub * P:(sub + 1) * P],
                        t_wv[:, dt, :],
                        start=(dt == 0), stop=(dt == DT - 1))
                nc.scalar.copy(
                    kv_sb[b][:, 0:HPC, kt0 + sub, DK:2 * DK], psv)
                if drip:
                    drip()

        def emit_attn(b):
            """Attention for both heads of batch b (flash-style, qc pairs)."""
            ot[b] = ot_pool.tile([P, HPC, SN], F16, tag="ot", name=f"ot{b}")
            rcp[b] = recip_pool.tile([P, HPC, NQC * NQC], F32, tag="recip", name=f"rcp{b}")
            pending = [None]
            with tc.tile_pool(name=f"pt{b}", bufs=6) as ptpool, \
                 tc.tile_pool(name=f"sm{b}", bufs=6) as smpool, \
                 tc.tile_pool(name=f"ps2_{b}", bufs=2, space="PSUM") as pp2, \
                 tc.tile_pool(name=f"po{b}", bufs=1, space="PSUM") as ppo:
                for h in range(HPC):
                    for pr in range(NQC // 2):
                        qA, qB = 2 * pr, 2 * pr + 1
                        q0 = qA * QCH
                        kts = list(range(SP // P + 4 * qB + 4))
                        po = ppo.tile([P, 2 * QCH], F32, tag="po")
                        acc = smpool.tile([P, 2 * QCH], F16, tag="acc")
                        state = {"fa": True, "fb": True}
                        pend = []

                        def a_valid(kt):
                            return kt < SP // P + 4 * qA + 4

                        def load_ktile(kt):
                            return (kv_sb[b][:, h, kt, 0:P],
                                    kv_sb[b][:, h, kt, DK:2 * DK])

                        def drain(last):
                            kt, ps_s, pt, v_t = pend.pop(0)
                            av = a_valid(kt)
                            ktn_i = kt - SP // P
                            oA, oB = ktn_i - 4 * qA, ktn_i - 4 * qB
                            soA = oA * P if 0 < oA < 4 else 0
                            soB = oB * P if 0 < oB < 4 else 0
                            lo = soA if av else QCH + soB
                            nc.scalar.activation(
                                pt[:, lo:], ps_s[:, lo:], EXP, scale=SCALE)
                            if 0 <= oA < 4:
                                nc.vector.tensor_mul(
                                    pt[:, soA:QCH], pt[:, soA:QCH],
                                    t_masks[:, oA, soA:])
                            if 0 <= oB < 4:
                                nc.vector.tensor_mul(
                                    pt[:, QCH + soB:], pt[:, QCH + soB:],
                                    t_masks[:, oB, soB:])
                            lastA = last or kt == SP // P + 4 * qA + 3
                            # accumulate probs for the softmax denominator on
                            # DVE (frees the tensor engine for scores/PV)
                            if av:
                                if state["fa"]:
                                    nc.vector.tensor_copy(acc, pt)
                                else:
                                    nc.vector.tensor_add(
                                        acc[:, lo:], acc[:, lo:], pt[:, lo:])
                            else:
                                nc.vector.tensor_add(
                                    acc[:, lo:], acc[:, lo:], pt[:, lo:])
                            if av:
                                nc.tensor.matmul(
                                    po[:, soA:QCH], v_t, pt[:, soA:QCH],
                                    start=state["fa"], stop=lastA,
                                    skip_group_check=True)
                                state["fa"] = False
                            nc.tensor.matmul(
                                po[:, QCH + soB:], v_t, pt[:, QCH + soB:],
                                start=state["fb"], stop=last,
                                skip_group_check=True)
                            state["fb"] = False

                        def sfx(kt, qc):
                            o = (kt - SP // P) - 4 * qc
                            return o * P if 0 < o < 4 else 0

                        for kt in kts:
                            if pending[0] is not None:
                                if kt == 3:
                                    pending[0][0]()
                                elif kt == 10:
                                    pending[0][1]()
                                    pending[0] = None
                            kt_t, v_t = load_ktile(kt)
                            ps_s = pp2.tile([P, 2 * QCH], F32, tag="ps_s")
                            pt = ptpool.tile([P, 2 * QCH], F16, tag="pt")
                            soA, soB = sfx(kt, qA), sfx(kt, qB)
                            if a_valid(kt):
                                nc.tensor.matmul(
                                    ps_s[:, soA:QCH], kt_t,
                                    qt[b][:, h, q0 + soA:q0 + QCH],
                                    start=True, stop=True)
                            nc.tensor.matmul(
                                ps_s[:, QCH + soB:], kt_t,
                                qt[b][:, h, q0 + QCH + soB:q0 + 2 * QCH],
                                start=True, stop=True)
                            pend.append((kt, ps_s, pt, v_t))
                            if len(pend) > 1:
                                drain(False)
                        drain(True)

                        nc.vector.tensor_copy(
                            ot[b][:, h, q0:q0 + QCH], po[:, 0:QCH])
                        nc.vector.tensor_copy(
                            ot[b][:, h, q0 + QCH:q0 + 2 * QCH], po[:, QCH:])

                        def make_fin(h=h, pr=pr, acc=acc):
                            st8 = {}

                            def fin_a():
                                ps_sum = ppo.tile([1, 2 * QCH], F32, tag="psum")
                                nc.tensor.matmul(
                                    ps_sum[0:1, 0:QCH], t_ones, acc[:, 0:QCH],
                                    start=True, stop=True)
                                nc.tensor.matmul(
                                    ps_sum[0:1, QCH:], t_ones, acc[:, QCH:],
                                    start=True, stop=True)
                                sb_sum = smpool.tile([1, 2 * QCH], F32, tag="sbsum")
                                nc.vector.tensor_copy(sb_sum, ps_sum)
                                st8["sb"] = sb_sum

                            def fin_b():
                                sb_sum = st8["sb"]
                                pst = ppo.tile([P, 8], F32, tag="psum")
                                for st in range(8):
                                    nc.tensor.matmul(
                                        pst[:, st:st + 1],
                                        sb_sum[0:1, st * P:(st + 1) * P],
                                        t_ones32[0:1, 0:1],
                                        start=True, stop=True)
                                nc.vector.reciprocal(
                                    rcp[b][:, h, pr * 8:(pr + 1) * 8], pst)
                            return fin_a, fin_b

                        pending[0] = make_fin()
                if pending[0] is not None:
                    pending[0][0]()
                    pending[0][1]()
                    pending[0] = None

        def emit_oproj_tile(b, st, ec, opool, ppso):
            """One Wo-projection output tile (with per-head normalisation)."""
            r0 = st * P
            e0 = ec * QCH
            pso0 = ppso.tile([P, QCH], F32, tag="pso")
            nc.tensor.matmul(
                pso0, ot[b][:, 0, r0:r0 + P],
                t_woT[:, 0, e0:e0 + QCH], start=True, stop=True)
            pso1 = ppso.tile([P, QCH], F32, tag="pso")
            nc.tensor.matmul(
                pso1, ot[b][:, 1, r0:r0 + P],
                t_woT[:, 1, e0:e0 + QCH], start=True, stop=True)
            tmp = opool.tile([P, QCH], F32, tag="tmp")
            nc.scalar.mul(tmp, pso0, rcp[b][:, 0, st:st + 1])
            outt = opool.tile([P, QCH], F16, tag="outt")
            eng = nc.vector
            eng.scalar_tensor_tensor(
                outt, pso1, rcp[b][:, 1, st:st + 1], tmp,
                op0=MULT, op1=ADD)
            nc.sync.dma_start(out[b, r0:r0 + P, e0:e0 + QCH], outt)

        def emit_oproj_group(b, sts, opool, ppso):
            for st in sts:
                for ec in range(D // QCH):
                    emit_oproj_tile(b, st, ec, opool, ppso)

        # ---------------- schedule ----------------
        def emit_ph1(b, xpool, pp1, ppv, drip=None, after_chunk0=None):
            for sc in range(NSC):
                emit_ph1_chunk(b, sc, xpool, pp1, ppv, drip)
                if sc == 0 and after_chunk0 is not None:
                    after_chunk0()
                # prefetch past-KV tiles into SBUF behind this chunk's x loads
                for h in range(HPC):
                    for kt in range(sc * 4, sc * 4 + 4):
                        nc.sync.dma_start(kv_sb[b][:, h, kt, :], pkv[b, h, kt])

        def load_consts():
            nc.sync.dma_start(t_masks, masks[:, :, :])
            nc.sync.dma_start(t_ones, ones_d[:, :])
            nc.sync.dma_start(t_ones32, ones32_d[:, :])

        qt[0] = qt_pool.tile([P, HPC, SN], F16, tag="qt", name="qt0")
        kv_sb[0] = kvsb_pool.tile([P, HPC, NKT, 2 * DK], F16, tag="kvsb", name="kvsb0")
        with tc.tile_pool(name="xt0", bufs=2) as xpool, \
             tc.tile_pool(name="ps1_0", bufs=3, space="PSUM") as pp1, \
             tc.tile_pool(name="pv0", bufs=2, space="PSUM") as ppv:
            emit_ph1(0, xpool, pp1, ppv, after_chunk0=load_consts)
        wor = woT.rearrange("(h p) d -> p h d", p=P)
        for hh in range(HPC):
            for piece in range(2):
                e0p = piece * (D // 2)
                nc.sync.dma_start(
                    t_woT[:, hh, e0p:e0p + D // 2], wor[:, hh, e0p:e0p + D // 2])

        emit_attn(0)

        qt[1] = qt_pool.tile([P, HPC, SN], F16, tag="qt", name="qt1")
        kv_sb[1] = kvsb_pool.tile([P, HPC, NKT, 2 * DK], F16, tag="kvsb", name="kvsb1")
        with tc.tile_pool(name="xt1", bufs=2) as xpool, \
             tc.tile_pool(name="ps1_1", bufs=2, space="PSUM") as pp1, \
             tc.tile_pool(name="pv1", bufs=2, space="PSUM") as ppv, \
             tc.tile_pool(name="os0", bufs=6) as opool, \
             tc.tile_pool(name="pso0", bufs=4, space="PSUM") as ppso:
            tiles0 = iter([(st, ec) for st in range(SN // P)
                           for ec in range(D // QCH)])

            def drip():
                for _ in range(2):
                    t = next(tiles0, None)
                    if t is not None:
                        emit_oproj_tile(0, t[0], t[1], opool, ppso)

            emit_ph1(1, xpool, pp1, ppv, drip)
            for st, ec in tiles0:
                emit_oproj_tile(0, st, ec, opool, ppso)

        emit_attn(1)

        with tc.tile_pool(name="os1", bufs=6) as opool, \
             tc.tile_pool(name="pso1", bufs=8, space="PSUM") as ppso:
            emit_oproj_group(1, range(SN // P), opool, ppso)

    nc.compile()
    return nc


def _get_nc():
    global _CACHED_NC
    if _CACHED_NC is None:
        _CACHED_NC = _build()
    return _CACHED_NC


def pack_kv(pk, pvv):
    # [B, HPC, S, DK] -> [B, HPC, S//P, P, 2*DK]: [.., 0:DK]=K^T tile, [.., DK:]=V tile
    b, hpc, s, dk = pk.shape
    kt = pk.reshape(b, hpc, s // P, P, dk).transpose(0, 1, 2, 4, 3)
    vt = pvv.reshape(b, hpc, s // P, P, dk)
    return np.ascontiguousarray(np.concatenate([kt, vt], axis=4))


def _prep_inputs(x, past_key, past_value, Wq, Wk, Wv, Wo):
    x = np.asarray(x, np.float32)
    past_key = np.asarray(past_key, np.float32)
    past_value = np.asarray(past_value, np.float32)
    Wq = np.asarray(Wq, np.float32)
    Wk = np.asarray(Wk, np.float32)
    Wv = np.asarray(Wv, np.float32)
    Wo = np.asarray(Wo, np.float32)

    xT = np.ascontiguousarray(x.transpose(0, 2, 1)).astype(np.float16)
    i = np.arange(P)[:, None]
    j = np.arange(QCH)[None, :]
    m = np.stack([(j >= i + o * P) for o in range(4)], axis=1).astype(np.float32)
    m = np.ascontiguousarray(m).astype(np.float16)  # [P, 4, QCH]
    ones = np.ones((P, 1), np.float16)
    ones32 = np.ones((1, 1), np.float32)

    in_maps = []
    for c in range(NCORES):
        e0 = c * EC
        hs = slice(c * HPC, (c + 1) * HPC)
        in_maps.append({
            "xT": xT,
            "wqT": np.ascontiguousarray(Wq[e0:e0 + EC, :].T).astype(np.float16),
            "wkT": np.ascontiguousarray(Wk[e0:e0 + EC, :].T).astype(np.float16),
            "wvT": np.ascontiguousarray(Wv[e0:e0 + EC, :].T).astype(np.float16),
            "woT": np.ascontiguousarray(Wo[:, e0:e0 + EC].T).astype(np.float16),
            "pkv": pack_kv(past_key[:, hs], past_value[:, hs]).astype(np.float16),
            "masks": m,
            "ones": ones,
            "ones32": ones32,
        })
    return in_maps


def _run(inputs, trace=False):
    nc = _get_nc()
    in_maps = _prep_inputs(**inputs)
    res = run_bass_kernel_spmd(nc, in_maps, core_ids=list(range(NCORES)), trace=trace)
    total = res.results[0]["out"].astype(np.float32)
    for c in range(1, NCORES):
        total += res.results[c]["out"]
    return total, res


def kernel(x, past_key, past_value, Wq, Wk, Wv, Wo):
    total, _ = _run(dict(x=x, past_key=past_key, past_value=past_value,
                         Wq=Wq, Wk=Wk, Wv=Wv, Wo=Wo))
    return total

